# revision 4
# baseline (speedup 1.0000x reference)
"""HB-LSTM cell fused Trainium2 kernel, data-parallel over 8 NeuronCores.

Computes, for gate order (f, i, o, u, k):
    pre  = x @ Wx[g].T + bx[g] + h_prev @ Uh[g].T + bh[g]
    f,i,o,u = sigmoid(pre[0..3]);  c = tanh(pre[4])
    kp = u*c + (1-u)*kp_prev
    k  = f*k_prev + i*kp
    h  = o*tanh(k)
Returns (h, k, kp), each [B, H] float32.

Sharding: batch dim B=65536 split across 8 cores (8192 rows each); the small
weight stacks are replicated to every core.
"""

import contextlib

import numpy as np

import concourse.bacc as bacc
import concourse.mybir as mybir
from concourse import tile
from concourse.bass_utils import run_bass_kernel_spmd

N_CORES = 8
B = 65536
IN = 256
H = 256
G5 = 5
BL = B // N_CORES          # rows per core
NT = BL // 128             # 64 b-tiles per core
GROUP = 4                  # b-tiles per DMA group
NG = NT // GROUP
DG = G5 * H                # 1280 = all-gate column span
F32 = mybir.dt.float32
BF16 = mybir.dt.bfloat16
AF = mybir.ActivationFunctionType

# Columns [0, PE_BIAS_COLS) get their bias from a K=1 ones-matmul on the PE;
# the rest get it from the DVE bias-add (balances PE vs DVE load).
PE_BIAS_COLS = 512

# Bench mode: when set, the main loop is wrapped in a hardware For_i loop
# running LOOP_N times so device time dominates RPC overhead in wall-clock.
LOOP_N = None

_CACHE = {}


def _build():
    if "nc" in _CACHE:
        return _CACHE["nc"]

    nc = bacc.Bacc("TRN2", target_bir_lowering=False, debug=False,
                   num_devices=N_CORES)

    x_d = nc.dram_tensor("x", [BL, IN], F32, kind="ExternalInput")
    h_d = nc.dram_tensor("h_prev", [BL, H], F32, kind="ExternalInput")
    k_d = nc.dram_tensor("k_prev", [BL, H], F32, kind="ExternalInput")
    kp_d = nc.dram_tensor("kp_prev", [BL, H], F32, kind="ExternalInput")
    wx_d = nc.dram_tensor("Wx", [G5, H, IN], F32, kind="ExternalInput")
    bx_d = nc.dram_tensor("bx", [G5, H], F32, kind="ExternalInput")
    uh_d = nc.dram_tensor("Uh", [G5, H, H], F32, kind="ExternalInput")
    bh_d = nc.dram_tensor("bh", [G5, H], F32, kind="ExternalInput")
    ho_d = nc.dram_tensor("h_out", [BL, H], F32, kind="ExternalOutput")
    ko_d = nc.dram_tensor("k_out", [BL, H], F32, kind="ExternalOutput")
    kpo_d = nc.dram_tensor("kp_out", [BL, H], F32, kind="ExternalOutput")

    with tile.TileContext(nc) as tc:
        with tc.tile_pool(name="const", bufs=1) as cpool:
            # --- weights: load fp32 -> bf16 (cast in DMA), transpose via xbar ---
            # WT[(side, c)]: [128 (i-chunk c), 1280 (g,h)] bf16, rhs of the matmuls
            WT = {}
            for side in ("x", "h"):
                for c in range(2):
                    WT[side, c] = cpool.tile([128, DG], BF16,
                                             name=f"WT_{side}{c}", tag=f"WT_{side}{c}")
            with tc.tile_pool(name="wload", bufs=2) as wload:
                for side, w_d in (("x", wx_d), ("h", uh_d)):
                    for g in range(G5):
                        w16 = wload.tile([128, 2, IN], BF16, tag="w16")
                        nc.gpsimd.dma_start(
                            w16[:],
                            w_d.ap()[g].rearrange("(hc p) i -> p hc i", p=128))
                        for c in range(2):
                            for hc in range(2):
                                col = g * H + hc * 128
                                nc.sync.dma_start(
                                    WT[side, c][:, col:col + 128],
                                    w16[:, hc, c * 128:(c + 1) * 128],
                                    transpose=True)

            # --- biases ---
            bs16 = cpool.tile([1, DG], BF16, tag="bs16")       # bf16 row for PE path
            biasb = cpool.tile([128, DG], F32, tag="biasb")    # broadcast for DVE path
            ones16 = cpool.tile([1, 128], BF16, tag="ones16")
            with tc.tile_pool(name="binit", bufs=1) as bpool, \
                 tc.tile_pool(name="binit_ps", bufs=1, space="PSUM") as bps:
                bxr = bpool.tile([1, DG], F32, tag="bxr")
                nc.sync.dma_start(bxr[:], bx_d.ap().rearrange("g h -> (g h)"))
                bhr = bpool.tile([1, DG], F32, tag="bhr")
                nc.sync.dma_start(bhr[:], bh_d.ap().rearrange("g h -> (g h)"))
                bsr = bpool.tile([1, DG], F32, tag="bsr")
                nc.vector.tensor_add(bsr[:], bxr[:], bhr[:])
                nc.vector.tensor_copy(bs16[:], bsr[:])
                nc.vector.memset(ones16[:], 1.0)
                psb = bps.tile([128, DG], F32, tag="psb")
                for n0 in range(0, DG, 512):
                    n1 = min(n0 + 512, DG)
                    nc.tensor.matmul(psb[:, n0:n1], ones16[:], bs16[:, n0:n1],
                                     start=True, stop=True)
                nc.vector.tensor_copy(biasb[:], psb[:])

            # --- main loop ---
            x_t = x_d.ap().rearrange("(n p) i -> p n i", p=128)
            h_t = h_d.ap().rearrange("(n p) i -> p n i", p=128)
            k_t = k_d.ap().rearrange("(n p) i -> p n i", p=128)
            kp_t = kp_d.ap().rearrange("(n p) i -> p n i", p=128)
            ho_t = ho_d.ap().rearrange("(n p) i -> p n i", p=128)
            ko_t = ko_d.ap().rearrange("(n p) i -> p n i", p=128)
            kpo_t = kpo_d.ap().rearrange("(n p) i -> p n i", p=128)

            loop_cm = (tc.For_i(0, LOOP_N, 1) if LOOP_N
                       else contextlib.nullcontext())
            with loop_cm, \
                 tc.tile_pool(name="io", bufs=2) as io, \
                 tc.tile_pool(name="work", bufs=3) as work, \
                 tc.tile_pool(name="psum", bufs=2, space="PSUM") as pp:
                for gi in range(NG):
                    nsl = slice(gi * GROUP, (gi + 1) * GROUP)
                    x16 = io.tile([128, GROUP, IN], BF16, tag="x16")
                    nc.gpsimd.dma_start(x16[:], x_t[:, nsl, :])   # casts f32->bf16
                    h16 = io.tile([128, GROUP, IN], BF16, tag="h16")
                    nc.gpsimd.dma_start(h16[:], h_t[:, nsl, :])
                    kpr = io.tile([128, GROUP, H], F32, tag="kpr")
                    nc.sync.dma_start(kpr[:], k_t[:, nsl, :])
                    kppr = io.tile([128, GROUP, H], F32, tag="kppr")
                    nc.sync.dma_start(kppr[:], kp_t[:, nsl, :])
                    kp_o = io.tile([128, GROUP, H], F32, tag="kp_o")
                    k_o = io.tile([128, GROUP, H], F32, tag="k_o")
                    h_o = io.tile([128, GROUP, H], F32, tag="h_o")

                    for j in range(GROUP):
                        xT = work.tile([128, IN], BF16, tag="xT")
                        hT = work.tile([128, IN], BF16, tag="hT")
                        for c in range(2):
                            nc.sync.dma_start(xT[:, c * 128:(c + 1) * 128],
                                              x16[:, j, c * 128:(c + 1) * 128],
                                              transpose=True)
                            nc.sync.dma_start(hT[:, c * 128:(c + 1) * 128],
                                              h16[:, j, c * 128:(c + 1) * 128],
                                              transpose=True)

                        ps = pp.tile([128, DG], F32, tag="ps")
                        nc.tensor.matmul(ps[:, 0:PE_BIAS_COLS], ones16[:],
                                         bs16[:, 0:PE_BIAS_COLS],
                                         start=True, stop=False)
                        for si, (side, aT) in enumerate((("x", xT), ("h", hT))):
                            for c in range(2):
                                lhsT = aT[:, c * 128:(c + 1) * 128]
                                for n0 in range(0, DG, 512):
                                    n1 = min(n0 + 512, DG)
                                    first = si == 0 and c == 0
                                    last = si == 1 and c == 1
                                    nc.tensor.matmul(
                                        ps[:, n0:n1], lhsT,
                                        WT[side, c][:, n0:n1],
                                        start=first and n0 >= PE_BIAS_COLS,
                                        stop=last)

                        # activations; bias for cols >= PE_BIAS_COLS added on DVE
                        fi = work.tile([128, 512], F32, tag="fi")
                        nc.scalar.activation(fi[:], ps[:, 0:512], AF.Sigmoid)
                        pre = work.tile([128, DG - 512], F32, tag="pre")
                        nc.vector.tensor_add(pre[:], ps[:, 512:DG],
                                             biasb[:, 512:DG])
                        ou = work.tile([128, 512], F32, tag="ou")
                        nc.scalar.activation(ou[:], pre[:, 0:512], AF.Sigmoid)
                        cg = work.tile([128, 256], F32, tag="cg")
                        nc.scalar.activation(cg[:], pre[:, 512:768], AF.Tanh)

                        f_ = fi[:, 0:256]
                        i_ = fi[:, 256:512]
                        o_ = ou[:, 0:256]
                        u_ = ou[:, 256:512]
                        kpp_j = kppr[:, j, :]
                        kpr_j = kpr[:, j, :]

                        d = work.tile([128, 256], F32, tag="d")
                        nc.gpsimd.tensor_sub(d[:], cg[:], kpp_j)
                        e = work.tile([128, 256], F32, tag="e")
                        nc.vector.tensor_mul(e[:], u_, d[:])
                        nc.vector.tensor_add(kp_o[:, j, :], e[:], kpp_j)
                        m = work.tile([128, 256], F32, tag="m")
                        nc.gpsimd.tensor_mul(m[:], f_, kpr_j)
                        n = work.tile([128, 256], F32, tag="n")
                        nc.vector.tensor_mul(n[:], i_, kp_o[:, j, :])
                        nc.vector.tensor_add(k_o[:, j, :], m[:], n[:])
                        tk = work.tile([128, 256], F32, tag="tk")
                        nc.scalar.activation(tk[:], k_o[:, j, :], AF.Tanh)
                        nc.gpsimd.tensor_mul(h_o[:, j, :], o_, tk[:])

                    nc.sync.dma_start(kpo_t[:, nsl, :], kp_o[:])
                    nc.sync.dma_start(ko_t[:, nsl, :], k_o[:])
                    nc.sync.dma_start(ho_t[:, nsl, :], h_o[:])

    nc.compile()
    _CACHE["nc"] = nc
    return nc


def kernel(x, h_prev, k_prev, kp_prev, Wx, bx, Uh, bh):
    x = np.asarray(x, dtype=np.float32)
    h_prev = np.asarray(h_prev, dtype=np.float32)
    k_prev = np.asarray(k_prev, dtype=np.float32)
    kp_prev = np.asarray(kp_prev, dtype=np.float32)
    Wx = np.ascontiguousarray(np.asarray(Wx, dtype=np.float32))
    bx = np.ascontiguousarray(np.asarray(bx, dtype=np.float32))
    Uh = np.ascontiguousarray(np.asarray(Uh, dtype=np.float32))
    bh = np.ascontiguousarray(np.asarray(bh, dtype=np.float32))

    nc = _build()
    in_maps = []
    for c in range(N_CORES):
        sl = slice(c * BL, (c + 1) * BL)
        in_maps.append({
            "x": np.ascontiguousarray(x[sl]),
            "h_prev": np.ascontiguousarray(h_prev[sl]),
            "k_prev": np.ascontiguousarray(k_prev[sl]),
            "kp_prev": np.ascontiguousarray(kp_prev[sl]),
            "Wx": Wx, "bx": bx, "Uh": Uh, "bh": bh,
        })
    res = run_bass_kernel_spmd(nc, in_maps, list(range(N_CORES)))
    h_out = np.concatenate([res.results[c]["h_out"] for c in range(N_CORES)], axis=0)
    k_out = np.concatenate([res.results[c]["k_out"] for c in range(N_CORES)], axis=0)
    kp_out = np.concatenate([res.results[c]["kp_out"] for c in range(N_CORES)], axis=0)
    return (h_out, k_out, kp_out)


# revision 7
# speedup vs baseline: 1.8992x; 1.8992x over previous
"""HB-LSTM cell fused Trainium2 kernel, data-parallel over 8 NeuronCores.

Computes, for gate order (f, i, o, u, k):
    pre  = x @ Wx[g].T + bx[g] + h_prev @ Uh[g].T + bh[g]
    f,i,o,u = sigmoid(pre[0..3]);  c = tanh(pre[4])
    kp = u*c + (1-u)*kp_prev
    k  = f*k_prev + i*kp
    h  = o*tanh(k)
Returns (h, k, kp), each [B, H] float32.

Sharding: batch dim B=65536 split across 8 cores (8192 rows each); weight
stacks replicated to every core.

Per-core structure (64 b-tiles of 128 rows):
  - x/h_prev loaded via SWDGE cast-DMA (fp32->bf16 in flight, Pool ring),
    staged c-major so ONE xbar DMA-transpose per (input, i-chunk, group)
    yields the feature-major lhsT tiles the PE needs.
  - 5-gate pre-activations accumulate in one [128,1280] PSUM tile per b-tile
    (12 bf16 matmuls + K=1 ones-matmul for part of the bias).
  - Sigmoid/Tanh on ACT straight out of PSUM; remaining bias via one fused
    DVE add; elementwise tail split between DVE and GPSIMD.
  - DMA issue spread across SP / ACT HWDGE rings and the Pool SWDGE ring.
"""

import contextlib

import numpy as np

import concourse.bacc as bacc
import concourse.mybir as mybir
from concourse import tile
from concourse.bass_utils import run_bass_kernel_spmd

N_CORES = 8
B = 65536
IN = 256
H = 256
G5 = 5
BL = B // N_CORES          # rows per core
NT = BL // 128             # 64 b-tiles per core
GROUP = 4                  # b-tiles per DMA group
NG = NT // GROUP
DG = G5 * H                # 1280 = all-gate column span
F32 = mybir.dt.float32
BF16 = mybir.dt.bfloat16
AF = mybir.ActivationFunctionType

# Gates [0, PE_BIAS_GATES) get bias from K=1 ones-matmuls on the PE; the rest
# from a fused DVE bias-add (balances PE vs DVE/ACT load). 0..5.
PE_BIAS_GATES = 2

# Engine for each elementwise op: "v" = DVE, "g" = GPSIMD.
OPS = {"d": "g", "e": "v", "kp": "v", "m": "g", "n": "v", "k": "v", "h": "g"}

# Bench mode: when set, the main loop runs LOOP_N times inside a hardware
# For_i loop so device time dominates RPC overhead in wall-clock.
LOOP_N = None

_CACHE = {}


def _build():
    if "nc" in _CACHE:
        return _CACHE["nc"]

    nc = bacc.Bacc("TRN2", target_bir_lowering=False, debug=False,
                   num_devices=N_CORES)

    x_d = nc.dram_tensor("x", [BL, IN], F32, kind="ExternalInput")
    h_d = nc.dram_tensor("h_prev", [BL, H], F32, kind="ExternalInput")
    k_d = nc.dram_tensor("k_prev", [BL, H], F32, kind="ExternalInput")
    kp_d = nc.dram_tensor("kp_prev", [BL, H], F32, kind="ExternalInput")
    wx_d = nc.dram_tensor("Wx", [G5, H, IN], F32, kind="ExternalInput")
    bx_d = nc.dram_tensor("bx", [G5, H], F32, kind="ExternalInput")
    uh_d = nc.dram_tensor("Uh", [G5, H, H], F32, kind="ExternalInput")
    bh_d = nc.dram_tensor("bh", [G5, H], F32, kind="ExternalInput")
    ho_d = nc.dram_tensor("h_out", [BL, H], F32, kind="ExternalOutput")
    ko_d = nc.dram_tensor("k_out", [BL, H], F32, kind="ExternalOutput")
    kpo_d = nc.dram_tensor("kp_out", [BL, H], F32, kind="ExternalOutput")

    vop = {"v": nc.vector, "g": nc.gpsimd}

    with tile.TileContext(nc) as tc:
        with tc.tile_pool(name="const", bufs=1) as cpool:
            # --- weights: fp32 -> bf16 (cast in DMA), i-major via xbar ---
            # WT[(side, c)]: [128 (i-chunk c), 1280 (g,h)] bf16 = matmul rhs
            WT = {}
            for side in ("x", "h"):
                for c in range(2):
                    WT[side, c] = cpool.tile([128, DG], BF16,
                                             name=f"WT_{side}{c}", tag=f"WT_{side}{c}")
            with tc.tile_pool(name="wload", bufs=2) as wload:
                for side, w_d in (("x", wx_d), ("h", uh_d)):
                    for g in range(G5):
                        w16 = wload.tile([128, 2, IN], BF16, tag="w16")
                        nc.gpsimd.dma_start(
                            w16[:],
                            w_d.ap()[g].rearrange("(hc p) i -> p hc i", p=128))
                        for c in range(2):
                            for hc in range(2):
                                col = g * H + hc * 128
                                nc.sync.dma_start(
                                    WT[side, c][:, col:col + 128],
                                    w16[:, hc, c * 128:(c + 1) * 128],
                                    transpose=True)

            # --- biases: bs16 [1,1280] bf16 row (PE path), biasb broadcast ---
            bs16 = cpool.tile([1, DG], BF16, tag="bs16")
            biasb = cpool.tile([128, DG], F32, tag="biasb")
            ones16 = cpool.tile([1, 128], BF16, tag="ones16")
            with tc.tile_pool(name="binit", bufs=1) as bpool, \
                 tc.tile_pool(name="binit_ps", bufs=1, space="PSUM") as bps:
                bxr = bpool.tile([G5, H], F32, tag="bxr")
                nc.sync.dma_start(bxr[:], bx_d.ap())
                bhr = bpool.tile([G5, H], F32, tag="bhr")
                nc.sync.dma_start(bhr[:], bh_d.ap())
                bsr = bpool.tile([G5, H], F32, tag="bsr")
                nc.vector.tensor_add(bsr[:], bxr[:], bhr[:])
                bsg = bpool.tile([G5, H], BF16, tag="bsg")
                nc.vector.tensor_copy(bsg[:], bsr[:])
                # flatten [5,256] -> one row [1,1280] (partition-major order)
                nc.sync.dma_start(bs16[:], bsg[:])
                nc.vector.memset(ones16[:], 1.0)
                psb = bps.tile([128, DG], F32, tag="psb")
                for n0 in range(0, DG, 512):
                    n1 = min(n0 + 512, DG)
                    nc.tensor.matmul(psb[:, n0:n1], ones16[:],
                                     bs16[:, n0:n1], start=True, stop=True)
                nc.vector.tensor_copy(biasb[:], psb[:])

            # --- main loop ---
            # c-major staging: [p, c, j(in group), q]
            x_cm = x_d.ap().rearrange("(n p) (c q) -> p c n q", p=128, q=128)
            h_cm = h_d.ap().rearrange("(n p) (c q) -> p c n q", p=128, q=128)
            k_t = k_d.ap().rearrange("(n p) i -> p n i", p=128)
            kp_t = kp_d.ap().rearrange("(n p) i -> p n i", p=128)
            ho_t = ho_d.ap().rearrange("(n p) i -> p n i", p=128)
            ko_t = ko_d.ap().rearrange("(n p) i -> p n i", p=128)
            kpo_t = kpo_d.ap().rearrange("(n p) i -> p n i", p=128)

            pe_cols = PE_BIAS_GATES * H
            loop_cm = (tc.For_i(0, LOOP_N, 1) if LOOP_N
                       else contextlib.nullcontext())
            with loop_cm, \
                 tc.tile_pool(name="io", bufs=2) as io, \
                 tc.tile_pool(name="work", bufs=4) as work, \
                 tc.tile_pool(name="psum", bufs=2, space="PSUM") as pp:
                for gi in range(NG):
                    nsl = slice(gi * GROUP, (gi + 1) * GROUP)
                    # SWDGE cast loads (Pool ring)
                    x16 = io.tile([128, 2, GROUP, 128], BF16, tag="x16")
                    nc.gpsimd.dma_start(x16[:], x_cm[:, :, nsl, :])
                    h16 = io.tile([128, 2, GROUP, 128], BF16, tag="h16")
                    nc.gpsimd.dma_start(h16[:], h_cm[:, :, nsl, :])
                    # fp32 state loads: kpr on SP ring, kppr on ACT ring
                    kpr = io.tile([128, GROUP, H], F32, tag="kpr")
                    nc.sync.dma_start(kpr[:], k_t[:, nsl, :])
                    kppr = io.tile([128, GROUP, H], F32, tag="kppr")
                    nc.scalar.dma_start(kppr[:], kp_t[:, nsl, :])
                    kp_o = io.tile([128, GROUP, H], F32, tag="kp_o")
                    k_o = io.tile([128, GROUP, H], F32, tag="k_o")
                    h_o = io.tile([128, GROUP, H], F32, tag="h_o")

                    # batched xbar transposes (SP ring): one per (input, c)
                    xT = work.tile([128, 2, GROUP, 128], BF16, tag="xT")
                    hT = work.tile([128, 2, GROUP, 128], BF16, tag="hT")
                    for c in range(2):
                        nc.sync.dma_start(xT[:, c], x16[:, c], transpose=True)
                        nc.sync.dma_start(hT[:, c], h16[:, c], transpose=True)

                    for j in range(GROUP):
                        ps = pp.tile([128, DG], F32, tag="ps")
                        for n0 in range(0, pe_cols, 512):
                            n1 = min(n0 + 512, pe_cols)
                            nc.tensor.matmul(ps[:, n0:n1],
                                             ones16[:], bs16[:, n0:n1],
                                             start=True, stop=False)
                        for si, (side, aT) in enumerate((("x", xT), ("h", hT))):
                            for c in range(2):
                                lhsT = aT[:, c, j, :]
                                for n0 in range(0, DG, 512):
                                    n1 = min(n0 + 512, DG)
                                    first = si == 0 and c == 0
                                    last = si == 1 and c == 1
                                    nc.tensor.matmul(
                                        ps[:, n0:n1], lhsT,
                                        WT[side, c][:, n0:n1],
                                        start=first and n0 >= pe_cols,
                                        stop=last)

                        # activations; bias for gates >= PE_BIAS_GATES on DVE
                        if pe_cols >= 1024:
                            gates = work.tile([128, 1024], F32, tag="gates")
                            nc.scalar.activation(gates[:], ps[:, 0:1024],
                                                 AF.Sigmoid)
                            cg = work.tile([128, 256], F32, tag="cg")
                            if pe_cols >= DG:
                                nc.scalar.activation(cg[:], ps[:, 1024:DG],
                                                     AF.Tanh)
                            else:
                                pre = work.tile([128, 256], F32, tag="pre")
                                nc.vector.tensor_add(pre[:], ps[:, 1024:DG],
                                                     biasb[:, 1024:DG])
                                nc.scalar.activation(cg[:], pre[:], AF.Tanh)
                            f_ = gates[:, 0:256]
                            i_ = gates[:, 256:512]
                            o_ = gates[:, 512:768]
                            u_ = gates[:, 768:1024]
                        else:
                            fi = work.tile([128, pe_cols], F32, tag="fi")
                            nc.scalar.activation(fi[:], ps[:, 0:pe_cols],
                                                 AF.Sigmoid)
                            pre = work.tile([128, DG - pe_cols], F32, tag="pre")
                            nc.vector.tensor_add(pre[:], ps[:, pe_cols:DG],
                                                 biasb[:, pe_cols:DG])
                            ou = work.tile([128, 1024 - pe_cols], F32, tag="ou")
                            nc.scalar.activation(ou[:], pre[:, 0:1024 - pe_cols],
                                                 AF.Sigmoid)
                            cg = work.tile([128, 256], F32, tag="cg")
                            nc.scalar.activation(
                                cg[:], pre[:, 1024 - pe_cols:DG - pe_cols],
                                AF.Tanh)
                            f_ = fi[:, 0:256]
                            i_ = fi[:, 256:512]
                            o_ = ou[:, 512 - pe_cols:768 - pe_cols]
                            u_ = ou[:, 768 - pe_cols:1024 - pe_cols]

                        kpp_j = kppr[:, j, :]
                        kpr_j = kpr[:, j, :]

                        d = work.tile([128, 256], F32, tag="d")
                        vop[OPS["d"]].tensor_sub(d[:], cg[:], kpp_j)
                        e = work.tile([128, 256], F32, tag="e")
                        vop[OPS["e"]].tensor_mul(e[:], u_, d[:])
                        vop[OPS["kp"]].tensor_add(kp_o[:, j, :], e[:], kpp_j)
                        m = work.tile([128, 256], F32, tag="m")
                        vop[OPS["m"]].tensor_mul(m[:], f_, kpr_j)
                        n = work.tile([128, 256], F32, tag="n")
                        vop[OPS["n"]].tensor_mul(n[:], i_, kp_o[:, j, :])
                        vop[OPS["k"]].tensor_add(k_o[:, j, :], m[:], n[:])
                        tk = work.tile([128, 256], F32, tag="tk")
                        nc.scalar.activation(tk[:], k_o[:, j, :], AF.Tanh)
                        vop[OPS["h"]].tensor_mul(h_o[:, j, :], o_, tk[:])

                    # stores: k,kp on SP ring; h on ACT ring
                    nc.sync.dma_start(kpo_t[:, nsl, :], kp_o[:])
                    nc.sync.dma_start(ko_t[:, nsl, :], k_o[:])
                    nc.scalar.dma_start(ho_t[:, nsl, :], h_o[:])

    nc.compile()
    _CACHE["nc"] = nc
    return nc


def kernel(x, h_prev, k_prev, kp_prev, Wx, bx, Uh, bh):
    x = np.asarray(x, dtype=np.float32)
    h_prev = np.asarray(h_prev, dtype=np.float32)
    k_prev = np.asarray(k_prev, dtype=np.float32)
    kp_prev = np.asarray(kp_prev, dtype=np.float32)
    Wx = np.ascontiguousarray(np.asarray(Wx, dtype=np.float32))
    bx = np.ascontiguousarray(np.asarray(bx, dtype=np.float32))
    Uh = np.ascontiguousarray(np.asarray(Uh, dtype=np.float32))
    bh = np.ascontiguousarray(np.asarray(bh, dtype=np.float32))

    nc = _build()
    in_maps = []
    for c in range(N_CORES):
        sl = slice(c * BL, (c + 1) * BL)
        in_maps.append({
            "x": np.ascontiguousarray(x[sl]),
            "h_prev": np.ascontiguousarray(h_prev[sl]),
            "k_prev": np.ascontiguousarray(k_prev[sl]),
            "kp_prev": np.ascontiguousarray(kp_prev[sl]),
            "Wx": Wx, "bx": bx, "Uh": Uh, "bh": bh,
        })
    res = run_bass_kernel_spmd(nc, in_maps, list(range(N_CORES)))
    h_out = np.concatenate([res.results[c]["h_out"] for c in range(N_CORES)], axis=0)
    k_out = np.concatenate([res.results[c]["k_out"] for c in range(N_CORES)], axis=0)
    kp_out = np.concatenate([res.results[c]["kp_out"] for c in range(N_CORES)], axis=0)
    return (h_out, k_out, kp_out)


# revision 9
# speedup vs baseline: 2.8169x; 1.4832x over previous
"""HB-LSTM cell fused Trainium2 kernel, data-parallel over 8 NeuronCores.

Computes, for gate order (f, i, o, u, k):
    pre  = x @ Wx[g].T + bx[g] + h_prev @ Uh[g].T + bh[g]
    f,i,o,u = sigmoid(pre[0..3]);  c = tanh(pre[4])
    kp = u*c + (1-u)*kp_prev
    k  = f*k_prev + i*kp
    h  = o*tanh(k)
Returns (h, k, kp), each [B, H] float32.

Sharding: batch dim B=65536 split across 8 cores (8192 rows each); weight
stacks replicated to every core.

Per-core structure (64 b-tiles of 128 rows):
  - x/h_prev loaded via SWDGE cast-DMA (fp32->bf16 in flight, Pool ring),
    staged c-major so ONE xbar DMA-transpose per (input, i-chunk, group)
    yields the feature-major lhsT tiles the PE needs.
  - 5-gate pre-activations accumulate in one [128,1280] PSUM tile per b-tile
    (12 bf16 matmuls + K=1 ones-matmul for part of the bias).
  - Sigmoid/Tanh on ACT straight out of PSUM; remaining bias via one fused
    DVE add; elementwise tail split between DVE and GPSIMD.
  - DMA issue spread across SP / ACT HWDGE rings and the Pool SWDGE ring.
"""

import contextlib

import numpy as np

import concourse.bacc as bacc
import concourse.mybir as mybir
from concourse import tile
from concourse.bass_utils import run_bass_kernel_spmd

N_CORES = 8
B = 65536
IN = 256
H = 256
G5 = 5
BL = B // N_CORES          # rows per core
NT = BL // 128             # 64 b-tiles per core
GROUP = 4                  # b-tiles per DMA group
NG = NT // GROUP
DG = G5 * H                # 1280 = all-gate column span
F32 = mybir.dt.float32
BF16 = mybir.dt.bfloat16
AF = mybir.ActivationFunctionType

# Gates [0, PE_BIAS_GATES) get bias from K=1 ones-matmuls on the PE; the rest
# from a fused DVE bias-add (balances PE vs DVE/ACT load). 0..5.
PE_BIAS_GATES = 5

# Engine for each elementwise op: "v" = DVE, "g" = GPSIMD.
OPS = {"d": "v", "e": "v", "kp": "v", "m": "v", "n": "v", "k": "v", "h": "v"}

# Bench mode: when set, the main loop runs LOOP_N times inside a hardware
# For_i loop so device time dominates RPC overhead in wall-clock.
LOOP_N = None

_CACHE = {}


def _build():
    if "nc" in _CACHE:
        return _CACHE["nc"]

    nc = bacc.Bacc("TRN2", target_bir_lowering=False, debug=False,
                   num_devices=N_CORES)

    x_d = nc.dram_tensor("x", [BL, IN], F32, kind="ExternalInput")
    h_d = nc.dram_tensor("h_prev", [BL, H], F32, kind="ExternalInput")
    k_d = nc.dram_tensor("k_prev", [BL, H], F32, kind="ExternalInput")
    kp_d = nc.dram_tensor("kp_prev", [BL, H], F32, kind="ExternalInput")
    wx_d = nc.dram_tensor("Wx", [G5, H, IN], F32, kind="ExternalInput")
    bx_d = nc.dram_tensor("bx", [G5, H], F32, kind="ExternalInput")
    uh_d = nc.dram_tensor("Uh", [G5, H, H], F32, kind="ExternalInput")
    bh_d = nc.dram_tensor("bh", [G5, H], F32, kind="ExternalInput")
    ho_d = nc.dram_tensor("h_out", [BL, H], F32, kind="ExternalOutput")
    ko_d = nc.dram_tensor("k_out", [BL, H], F32, kind="ExternalOutput")
    kpo_d = nc.dram_tensor("kp_out", [BL, H], F32, kind="ExternalOutput")

    vop = {"v": nc.vector, "g": nc.gpsimd}

    with tile.TileContext(nc) as tc:
        with tc.tile_pool(name="const", bufs=1) as cpool:
            # --- weights: fp32 -> bf16 (cast in DMA), i-major via xbar ---
            # WT[(side, c)]: [128 (i-chunk c), 1280 (g,h)] bf16 = matmul rhs
            WT = {}
            for side in ("x", "h"):
                for c in range(2):
                    WT[side, c] = cpool.tile([128, DG], BF16,
                                             name=f"WT_{side}{c}", tag=f"WT_{side}{c}")
            with tc.tile_pool(name="wload", bufs=2) as wload:
                for side, w_d in (("x", wx_d), ("h", uh_d)):
                    for g in range(G5):
                        w16 = wload.tile([128, 2, IN], BF16, tag="w16")
                        nc.gpsimd.dma_start(
                            w16[:],
                            w_d.ap()[g].rearrange("(hc p) i -> p hc i", p=128))
                        for c in range(2):
                            for hc in range(2):
                                col = g * H + hc * 128
                                nc.sync.dma_start(
                                    WT[side, c][:, col:col + 128],
                                    w16[:, hc, c * 128:(c + 1) * 128],
                                    transpose=True)

            # --- biases: bs16 [1,1280] bf16 row (PE path), biasb broadcast ---
            bs16 = cpool.tile([1, DG], BF16, tag="bs16")
            biasb = cpool.tile([128, DG], F32, tag="biasb")
            ones16 = cpool.tile([1, 128], BF16, tag="ones16")
            with tc.tile_pool(name="binit", bufs=1) as bpool, \
                 tc.tile_pool(name="binit_ps", bufs=1, space="PSUM") as bps:
                bxr = bpool.tile([G5, H], F32, tag="bxr")
                nc.sync.dma_start(bxr[:], bx_d.ap())
                bhr = bpool.tile([G5, H], F32, tag="bhr")
                nc.sync.dma_start(bhr[:], bh_d.ap())
                bsr = bpool.tile([G5, H], F32, tag="bsr")
                nc.vector.tensor_add(bsr[:], bxr[:], bhr[:])
                bsg = bpool.tile([G5, H], BF16, tag="bsg")
                nc.vector.tensor_copy(bsg[:], bsr[:])
                # flatten [5,256] -> one row [1,1280] (partition-major order)
                nc.sync.dma_start(bs16[:], bsg[:])
                nc.vector.memset(ones16[:], 1.0)
                psb = bps.tile([128, DG], F32, tag="psb")
                for n0 in range(0, DG, 512):
                    n1 = min(n0 + 512, DG)
                    nc.tensor.matmul(psb[:, n0:n1], ones16[:],
                                     bs16[:, n0:n1], start=True, stop=True)
                nc.vector.tensor_copy(biasb[:], psb[:])

            # --- main loop ---
            # c-major staging: [p, c, j(in group), q]
            x_cm = x_d.ap().rearrange("(n p) (c q) -> p c n q", p=128, q=128)
            h_cm = h_d.ap().rearrange("(n p) (c q) -> p c n q", p=128, q=128)
            k_t = k_d.ap().rearrange("(n p) i -> p n i", p=128)
            kp_t = kp_d.ap().rearrange("(n p) i -> p n i", p=128)
            ho_t = ho_d.ap().rearrange("(n p) i -> p n i", p=128)
            ko_t = ko_d.ap().rearrange("(n p) i -> p n i", p=128)
            kpo_t = kpo_d.ap().rearrange("(n p) i -> p n i", p=128)

            pe_cols = PE_BIAS_GATES * H
            loop_cm = (tc.For_i(0, LOOP_N, 1) if LOOP_N
                       else contextlib.nullcontext())
            with tc.tile_pool(name="io", bufs=2) as io, \
                 tc.tile_pool(name="work", bufs=4) as work, \
                 tc.tile_pool(name="psum", bufs=2, space="PSUM") as pp, \
                 loop_cm:
                for gi in range(NG):
                    nsl = slice(gi * GROUP, (gi + 1) * GROUP)
                    # SWDGE cast loads (Pool ring)
                    x16 = io.tile([128, 2, GROUP, 128], BF16, tag="x16")
                    nc.gpsimd.dma_start(x16[:], x_cm[:, :, nsl, :])
                    h16 = io.tile([128, 2, GROUP, 128], BF16, tag="h16")
                    nc.gpsimd.dma_start(h16[:], h_cm[:, :, nsl, :])
                    # fp32 state loads: kpr on SP ring, kppr on ACT ring
                    kpr = io.tile([128, GROUP, H], F32, tag="kpr")
                    nc.sync.dma_start(kpr[:], k_t[:, nsl, :])
                    kppr = io.tile([128, GROUP, H], F32, tag="kppr")
                    nc.scalar.dma_start(kppr[:], kp_t[:, nsl, :])
                    kp_o = io.tile([128, GROUP, H], F32, tag="kp_o")
                    k_o = io.tile([128, GROUP, H], F32, tag="k_o")
                    h_o = io.tile([128, GROUP, H], F32, tag="h_o")

                    # batched xbar transposes (SP ring): one per (input, c)
                    xT = work.tile([128, 2, GROUP, 128], BF16, tag="xT")
                    hT = work.tile([128, 2, GROUP, 128], BF16, tag="hT")
                    for c in range(2):
                        nc.sync.dma_start(xT[:, c], x16[:, c], transpose=True)
                        nc.sync.dma_start(hT[:, c], h16[:, c], transpose=True)

                    for j in range(GROUP):
                        ps = pp.tile([128, DG], F32, tag="ps")
                        for n0 in range(0, pe_cols, 512):
                            n1 = min(n0 + 512, pe_cols)
                            nc.tensor.matmul(ps[:, n0:n1],
                                             ones16[:], bs16[:, n0:n1],
                                             start=True, stop=False)
                        for si, (side, aT) in enumerate((("x", xT), ("h", hT))):
                            for c in range(2):
                                lhsT = aT[:, c, j, :]
                                for n0 in range(0, DG, 512):
                                    n1 = min(n0 + 512, DG)
                                    first = si == 0 and c == 0
                                    last = si == 1 and c == 1
                                    nc.tensor.matmul(
                                        ps[:, n0:n1], lhsT,
                                        WT[side, c][:, n0:n1],
                                        start=first and n0 >= pe_cols,
                                        stop=last)

                        # activations; bias for gates >= PE_BIAS_GATES on DVE
                        if pe_cols >= 1024:
                            gates = work.tile([128, 1024], F32, tag="gates")
                            nc.scalar.activation(gates[:], ps[:, 0:1024],
                                                 AF.Sigmoid)
                            cg = work.tile([128, 256], F32, tag="cg")
                            if pe_cols >= DG:
                                nc.scalar.activation(cg[:], ps[:, 1024:DG],
                                                     AF.Tanh)
                            else:
                                pre = work.tile([128, 256], F32, tag="pre")
                                nc.vector.tensor_add(pre[:], ps[:, 1024:DG],
                                                     biasb[:, 1024:DG])
                                nc.scalar.activation(cg[:], pre[:], AF.Tanh)
                            f_ = gates[:, 0:256]
                            i_ = gates[:, 256:512]
                            o_ = gates[:, 512:768]
                            u_ = gates[:, 768:1024]
                        else:
                            fi = work.tile([128, pe_cols], F32, tag="fi")
                            nc.scalar.activation(fi[:], ps[:, 0:pe_cols],
                                                 AF.Sigmoid)
                            pre = work.tile([128, DG - pe_cols], F32, tag="pre")
                            nc.vector.tensor_add(pre[:], ps[:, pe_cols:DG],
                                                 biasb[:, pe_cols:DG])
                            ou = work.tile([128, 1024 - pe_cols], F32, tag="ou")
                            nc.scalar.activation(ou[:], pre[:, 0:1024 - pe_cols],
                                                 AF.Sigmoid)
                            cg = work.tile([128, 256], F32, tag="cg")
                            nc.scalar.activation(
                                cg[:], pre[:, 1024 - pe_cols:DG - pe_cols],
                                AF.Tanh)
                            f_ = fi[:, 0:256]
                            i_ = fi[:, 256:512]
                            o_ = ou[:, 512 - pe_cols:768 - pe_cols]
                            u_ = ou[:, 768 - pe_cols:1024 - pe_cols]

                        kpp_j = kppr[:, j, :]
                        kpr_j = kpr[:, j, :]

                        d = work.tile([128, 256], F32, tag="d")
                        vop[OPS["d"]].tensor_sub(d[:], cg[:], kpp_j)
                        e = work.tile([128, 256], F32, tag="e")
                        vop[OPS["e"]].tensor_mul(e[:], u_, d[:])
                        vop[OPS["kp"]].tensor_add(kp_o[:, j, :], e[:], kpp_j)
                        m = work.tile([128, 256], F32, tag="m")
                        vop[OPS["m"]].tensor_mul(m[:], f_, kpr_j)
                        n = work.tile([128, 256], F32, tag="n")
                        vop[OPS["n"]].tensor_mul(n[:], i_, kp_o[:, j, :])
                        vop[OPS["k"]].tensor_add(k_o[:, j, :], m[:], n[:])
                        tk = work.tile([128, 256], F32, tag="tk")
                        nc.scalar.activation(tk[:], k_o[:, j, :], AF.Tanh)
                        vop[OPS["h"]].tensor_mul(h_o[:, j, :], o_, tk[:])

                    # stores: k,kp on SP ring; h on ACT ring
                    nc.sync.dma_start(kpo_t[:, nsl, :], kp_o[:])
                    nc.sync.dma_start(ko_t[:, nsl, :], k_o[:])
                    nc.scalar.dma_start(ho_t[:, nsl, :], h_o[:])

    nc.compile()
    _CACHE["nc"] = nc
    return nc


def kernel(x, h_prev, k_prev, kp_prev, Wx, bx, Uh, bh):
    x = np.asarray(x, dtype=np.float32)
    h_prev = np.asarray(h_prev, dtype=np.float32)
    k_prev = np.asarray(k_prev, dtype=np.float32)
    kp_prev = np.asarray(kp_prev, dtype=np.float32)
    Wx = np.ascontiguousarray(np.asarray(Wx, dtype=np.float32))
    bx = np.ascontiguousarray(np.asarray(bx, dtype=np.float32))
    Uh = np.ascontiguousarray(np.asarray(Uh, dtype=np.float32))
    bh = np.ascontiguousarray(np.asarray(bh, dtype=np.float32))

    nc = _build()
    in_maps = []
    for c in range(N_CORES):
        sl = slice(c * BL, (c + 1) * BL)
        in_maps.append({
            "x": np.ascontiguousarray(x[sl]),
            "h_prev": np.ascontiguousarray(h_prev[sl]),
            "k_prev": np.ascontiguousarray(k_prev[sl]),
            "kp_prev": np.ascontiguousarray(kp_prev[sl]),
            "Wx": Wx, "bx": bx, "Uh": Uh, "bh": bh,
        })
    res = run_bass_kernel_spmd(nc, in_maps, list(range(N_CORES)))
    h_out = np.concatenate([res.results[c]["h_out"] for c in range(N_CORES)], axis=0)
    k_out = np.concatenate([res.results[c]["k_out"] for c in range(N_CORES)], axis=0)
    kp_out = np.concatenate([res.results[c]["kp_out"] for c in range(N_CORES)], axis=0)
    return (h_out, k_out, kp_out)


# revision 11
# speedup vs baseline: 3.2087x; 1.1391x over previous
"""HB-LSTM cell fused Trainium2 kernel, data-parallel over 8 NeuronCores.

Computes, for gate order (f, i, o, u, k):
    pre  = x @ Wx[g].T + bx[g] + h_prev @ Uh[g].T + bh[g]
    f,i,o,u = sigmoid(pre[0..3]);  c = tanh(pre[4])
    kp = u*c + (1-u)*kp_prev
    k  = f*k_prev + i*kp
    h  = o*tanh(k)
Returns (h, k, kp), each [B, H] float32.

Sharding: batch dim B=65536 split across 8 cores (8192 rows each); weight
stacks replicated to every core.

Per-core structure (64 b-tiles of 128 rows):
  - x/h_prev loaded via SWDGE cast-DMA (fp32->bf16 in flight, Pool ring),
    staged c-major so ONE xbar DMA-transpose per (input, i-chunk, group)
    yields the feature-major lhsT tiles the PE needs.
  - 5-gate pre-activations accumulate in one [128,1280] PSUM tile per b-tile
    (12 bf16 matmuls + K=1 ones-matmul for part of the bias).
  - Sigmoid/Tanh on ACT straight out of PSUM; remaining bias via one fused
    DVE add; elementwise tail split between DVE and GPSIMD.
  - DMA issue spread across SP / ACT HWDGE rings and the Pool SWDGE ring.
"""

import contextlib

import numpy as np

import concourse.bacc as bacc
import concourse.mybir as mybir
from concourse import tile
from concourse.bass_utils import run_bass_kernel_spmd

N_CORES = 8
B = 65536
IN = 256
H = 256
G5 = 5
BL = B // N_CORES          # rows per core
NT = BL // 128             # 64 b-tiles per core
GROUP = 4                  # b-tiles per DMA group
NG = NT // GROUP
DG = G5 * H                # 1280 = all-gate column span
F32 = mybir.dt.float32
BF16 = mybir.dt.bfloat16
AF = mybir.ActivationFunctionType

# Gates [0, PE_BIAS_GATES) get bias from K=1 ones-matmuls on the PE; the rest
# from a fused DVE bias-add (balances PE vs DVE/ACT load). 0..5.
PE_BIAS_GATES = 5

# Engine for each elementwise op: "v" = DVE, "g" = GPSIMD.
OPS = {"d": "v", "e": "v", "kp": "v", "m": "v", "n": "v", "k": "v", "h": "v"}

# Bench mode: when set, the main loop runs LOOP_N times inside a hardware
# For_i loop so device time dominates RPC overhead in wall-clock.
LOOP_N = None

# Probe mode for HW decomposition benches: None = full kernel,
# "pe" = input loads + transposes + matmuls only (no ACT/DVE/stores).
PROBE = None

_CACHE = {}


def _build():
    if "nc" in _CACHE:
        return _CACHE["nc"]

    nc = bacc.Bacc("TRN2", target_bir_lowering=False, debug=False,
                   num_devices=N_CORES)

    x_d = nc.dram_tensor("x", [BL, IN], F32, kind="ExternalInput")
    h_d = nc.dram_tensor("h_prev", [BL, H], F32, kind="ExternalInput")
    k_d = nc.dram_tensor("k_prev", [BL, H], F32, kind="ExternalInput")
    kp_d = nc.dram_tensor("kp_prev", [BL, H], F32, kind="ExternalInput")
    wx_d = nc.dram_tensor("Wx", [G5, H, IN], F32, kind="ExternalInput")
    bx_d = nc.dram_tensor("bx", [G5, H], F32, kind="ExternalInput")
    uh_d = nc.dram_tensor("Uh", [G5, H, H], F32, kind="ExternalInput")
    bh_d = nc.dram_tensor("bh", [G5, H], F32, kind="ExternalInput")
    ho_d = nc.dram_tensor("h_out", [BL, H], F32, kind="ExternalOutput")
    ko_d = nc.dram_tensor("k_out", [BL, H], F32, kind="ExternalOutput")
    kpo_d = nc.dram_tensor("kp_out", [BL, H], F32, kind="ExternalOutput")

    vop = {"v": nc.vector, "g": nc.gpsimd}

    with tile.TileContext(nc) as tc:
        with tc.tile_pool(name="const", bufs=1) as cpool:
            # --- weights: fp32 -> bf16 (cast in DMA), i-major via xbar ---
            # WT[(side, c)]: [128 (i-chunk c), 1280 (g,h)] bf16 = matmul rhs
            WT = {}
            for side in ("x", "h"):
                for c in range(2):
                    WT[side, c] = cpool.tile([128, DG], BF16,
                                             name=f"WT_{side}{c}", tag=f"WT_{side}{c}")
            with tc.tile_pool(name="wload", bufs=2) as wload:
                for side, w_d in (("x", wx_d), ("h", uh_d)):
                    for g in range(G5):
                        w16 = wload.tile([128, 2, IN], BF16, tag="w16")
                        nc.gpsimd.dma_start(
                            w16[:],
                            w_d.ap()[g].rearrange("(hc p) i -> p hc i", p=128))
                        for c in range(2):
                            for hc in range(2):
                                col = g * H + hc * 128
                                nc.sync.dma_start(
                                    WT[side, c][:, col:col + 128],
                                    w16[:, hc, c * 128:(c + 1) * 128],
                                    transpose=True)

            # --- biases: bs16 [1,1280] bf16 row (PE path), biasb broadcast ---
            bs16 = cpool.tile([1, DG], BF16, tag="bs16")
            biasb = cpool.tile([128, DG], F32, tag="biasb")
            ones16 = cpool.tile([1, 128], BF16, tag="ones16")
            with tc.tile_pool(name="binit", bufs=1) as bpool, \
                 tc.tile_pool(name="binit_ps", bufs=1, space="PSUM") as bps:
                bxr = bpool.tile([G5, H], F32, tag="bxr")
                nc.sync.dma_start(bxr[:], bx_d.ap())
                bhr = bpool.tile([G5, H], F32, tag="bhr")
                nc.sync.dma_start(bhr[:], bh_d.ap())
                bsr = bpool.tile([G5, H], F32, tag="bsr")
                nc.vector.tensor_add(bsr[:], bxr[:], bhr[:])
                bsg = bpool.tile([G5, H], BF16, tag="bsg")
                nc.vector.tensor_copy(bsg[:], bsr[:])
                # flatten [5,256] -> one row [1,1280] (partition-major order)
                nc.sync.dma_start(bs16[:], bsg[:])
                nc.vector.memset(ones16[:], 1.0)
                psb = bps.tile([128, DG], F32, tag="psb")
                for n0 in range(0, DG, 512):
                    n1 = min(n0 + 512, DG)
                    nc.tensor.matmul(psb[:, n0:n1], ones16[:],
                                     bs16[:, n0:n1], start=True, stop=True)
                nc.vector.tensor_copy(biasb[:], psb[:])

            # --- main loop ---
            # c-major staging: [p, c, j(in group), q]
            x_cm = x_d.ap().rearrange("(n p) (c q) -> p c n q", p=128, q=128)
            h_cm = h_d.ap().rearrange("(n p) (c q) -> p c n q", p=128, q=128)
            k_t = k_d.ap().rearrange("(n p) i -> p n i", p=128)
            kp_t = kp_d.ap().rearrange("(n p) i -> p n i", p=128)
            ho_t = ho_d.ap().rearrange("(n p) i -> p n i", p=128)
            ko_t = ko_d.ap().rearrange("(n p) i -> p n i", p=128)
            kpo_t = kpo_d.ap().rearrange("(n p) i -> p n i", p=128)

            pe_cols = PE_BIAS_GATES * H
            loop_cm = (tc.For_i(0, LOOP_N, 1) if LOOP_N
                       else contextlib.nullcontext())
            with tc.tile_pool(name="io", bufs=2) as io, \
                 tc.tile_pool(name="work", bufs=4) as work, \
                 tc.tile_pool(name="psum", bufs=2, space="PSUM") as pp, \
                 loop_cm:
                for gi in range(NG):
                    nsl = slice(gi * GROUP, (gi + 1) * GROUP)
                    # SWDGE cast loads (Pool ring)
                    x16 = io.tile([128, 2, GROUP, 128], BF16, tag="x16")
                    nc.gpsimd.dma_start(x16[:], x_cm[:, :, nsl, :])
                    h16 = io.tile([128, 2, GROUP, 128], BF16, tag="h16")
                    nc.gpsimd.dma_start(h16[:], h_cm[:, :, nsl, :])
                    # fp32 state loads: kpr on SP ring, kppr on ACT ring
                    if PROBE != "pe":
                        kpr = io.tile([128, GROUP, H], F32, tag="kpr")
                        nc.sync.dma_start(kpr[:], k_t[:, nsl, :])
                        kppr = io.tile([128, GROUP, H], F32, tag="kppr")
                        nc.scalar.dma_start(kppr[:], kp_t[:, nsl, :])
                        kp_o = io.tile([128, GROUP, H], F32, tag="kp_o")
                        k_o = io.tile([128, GROUP, H], F32, tag="k_o")
                        h_o = io.tile([128, GROUP, H], F32, tag="h_o")

                    # batched xbar transposes (SP ring): one per (input, c)
                    xT = work.tile([128, 2, GROUP, 128], BF16, tag="xT")
                    hT = work.tile([128, 2, GROUP, 128], BF16, tag="hT")
                    for c in range(2):
                        nc.sync.dma_start(xT[:, c], x16[:, c], transpose=True)
                        nc.sync.dma_start(hT[:, c], h16[:, c], transpose=True)

                    for j in range(GROUP):
                        ps = pp.tile([128, DG], F32, tag="ps")
                        for n0 in range(0, pe_cols, 512):
                            n1 = min(n0 + 512, pe_cols)
                            nc.tensor.matmul(ps[:, n0:n1],
                                             ones16[:], bs16[:, n0:n1],
                                             start=True, stop=False)
                        for si, (side, aT) in enumerate((("x", xT), ("h", hT))):
                            for c in range(2):
                                lhsT = aT[:, c, j, :]
                                for n0 in range(0, DG, 512):
                                    n1 = min(n0 + 512, DG)
                                    first = si == 0 and c == 0
                                    last = si == 1 and c == 1
                                    nc.tensor.matmul(
                                        ps[:, n0:n1], lhsT,
                                        WT[side, c][:, n0:n1],
                                        start=first and n0 >= pe_cols,
                                        stop=last)

                        if PROBE == "pe":
                            continue
                        # activations; bias for gates >= PE_BIAS_GATES on DVE
                        if pe_cols >= 1024:
                            gates = work.tile([128, 1024], F32, tag="gates")
                            nc.scalar.activation(gates[:], ps[:, 0:1024],
                                                 AF.Sigmoid)
                            cg = work.tile([128, 256], F32, tag="cg")
                            if pe_cols >= DG:
                                nc.scalar.activation(cg[:], ps[:, 1024:DG],
                                                     AF.Tanh)
                            else:
                                pre = work.tile([128, 256], F32, tag="pre")
                                nc.vector.tensor_add(pre[:], ps[:, 1024:DG],
                                                     biasb[:, 1024:DG])
                                nc.scalar.activation(cg[:], pre[:], AF.Tanh)
                            f_ = gates[:, 0:256]
                            i_ = gates[:, 256:512]
                            o_ = gates[:, 512:768]
                            u_ = gates[:, 768:1024]
                        else:
                            fi = work.tile([128, pe_cols], F32, tag="fi")
                            nc.scalar.activation(fi[:], ps[:, 0:pe_cols],
                                                 AF.Sigmoid)
                            pre = work.tile([128, DG - pe_cols], F32, tag="pre")
                            nc.vector.tensor_add(pre[:], ps[:, pe_cols:DG],
                                                 biasb[:, pe_cols:DG])
                            ou = work.tile([128, 1024 - pe_cols], F32, tag="ou")
                            nc.scalar.activation(ou[:], pre[:, 0:1024 - pe_cols],
                                                 AF.Sigmoid)
                            cg = work.tile([128, 256], F32, tag="cg")
                            nc.scalar.activation(
                                cg[:], pre[:, 1024 - pe_cols:DG - pe_cols],
                                AF.Tanh)
                            f_ = fi[:, 0:256]
                            i_ = fi[:, 256:512]
                            o_ = ou[:, 512 - pe_cols:768 - pe_cols]
                            u_ = ou[:, 768 - pe_cols:1024 - pe_cols]

                        kpp_j = kppr[:, j, :]
                        kpr_j = kpr[:, j, :]

                        d = work.tile([128, 256], F32, tag="d")
                        vop[OPS["d"]].tensor_sub(d[:], cg[:], kpp_j)
                        e = work.tile([128, 256], F32, tag="e")
                        vop[OPS["e"]].tensor_mul(e[:], u_, d[:])
                        vop[OPS["kp"]].tensor_add(kp_o[:, j, :], e[:], kpp_j)
                        m = work.tile([128, 256], F32, tag="m")
                        vop[OPS["m"]].tensor_mul(m[:], f_, kpr_j)
                        n = work.tile([128, 256], F32, tag="n")
                        vop[OPS["n"]].tensor_mul(n[:], i_, kp_o[:, j, :])
                        vop[OPS["k"]].tensor_add(k_o[:, j, :], m[:], n[:])
                        tk = work.tile([128, 256], F32, tag="tk")
                        nc.scalar.activation(tk[:], k_o[:, j, :], AF.Tanh)
                        vop[OPS["h"]].tensor_mul(h_o[:, j, :], o_, tk[:])

                    # stores: k,kp on SP ring; h on ACT ring
                    if PROBE != "pe":
                        nc.sync.dma_start(kpo_t[:, nsl, :], kp_o[:])
                        nc.sync.dma_start(ko_t[:, nsl, :], k_o[:])
                        nc.scalar.dma_start(ho_t[:, nsl, :], h_o[:])

    nc.compile()
    _CACHE["nc"] = nc
    return nc


def kernel(x, h_prev, k_prev, kp_prev, Wx, bx, Uh, bh):
    x = np.asarray(x, dtype=np.float32)
    h_prev = np.asarray(h_prev, dtype=np.float32)
    k_prev = np.asarray(k_prev, dtype=np.float32)
    kp_prev = np.asarray(kp_prev, dtype=np.float32)
    Wx = np.ascontiguousarray(np.asarray(Wx, dtype=np.float32))
    bx = np.ascontiguousarray(np.asarray(bx, dtype=np.float32))
    Uh = np.ascontiguousarray(np.asarray(Uh, dtype=np.float32))
    bh = np.ascontiguousarray(np.asarray(bh, dtype=np.float32))

    nc = _build()
    in_maps = []
    for c in range(N_CORES):
        sl = slice(c * BL, (c + 1) * BL)
        in_maps.append({
            "x": np.ascontiguousarray(x[sl]),
            "h_prev": np.ascontiguousarray(h_prev[sl]),
            "k_prev": np.ascontiguousarray(k_prev[sl]),
            "kp_prev": np.ascontiguousarray(kp_prev[sl]),
            "Wx": Wx, "bx": bx, "Uh": Uh, "bh": bh,
        })
    res = run_bass_kernel_spmd(nc, in_maps, list(range(N_CORES)))
    h_out = np.concatenate([res.results[c]["h_out"] for c in range(N_CORES)], axis=0)
    k_out = np.concatenate([res.results[c]["k_out"] for c in range(N_CORES)], axis=0)
    kp_out = np.concatenate([res.results[c]["kp_out"] for c in range(N_CORES)], axis=0)
    return (h_out, k_out, kp_out)
